# revision 35
# baseline (speedup 1.0000x reference)
"""DWARF attention Trainium2 Bass kernel (v3, bf16 + pipelined halves).

Sharding: 8 cores = 4 batches x 2 head-halves (8 local heads each).
Per-core dataflow (feature-major = [feature rows, token cols]):
  P1 proj:  q/k/v/gate = W^T.T @ xT on PE (bf16), ACT evictions w/ bias+sigmoid
            k/v evicted into left-zero-padded resident tiles (shifted reads)
  P2 E:     E = exp(q_offset + prior) token-major (bf16), SE row-sums (f32),
            EC = E*coef tap table built off critical path
  P3 qk:    per offset: 4 pair-products (DVE bf16) -> 16 pair-sum matmuls into
            one 4-bank psum tile at row bases {0,32,64,96} -> 4 direct
            psum->SBUF row DMAs into qk_T [88,N] f32
  P4 tm:    PE-transpose qk_T -> qk_tm token-major bf16
  P5 feat:  (per token-half) tap-gathers, feat=elu(qk+b)+1, A=EC*feat/z
  P6 A_T:   (per half) PE-transpose A_tm (f32) -> A_stage [88,N] bf16
  P7 AV:    (per nb) per (pair,off): sel-matmul expand, DVE mul w/ padded v2,
            PE identity accumulate
  P8 out:   (per nb) gg = out_fm*gate; y_fm = Wout^T.T @ gg (PE bf16) -> DRAM
Host: shard, pre-transpose weights to bf16, build sel/tap tables, reduce
head-halves.
"""
from contextlib import ExitStack

import ml_dtypes
import numpy as np

import concourse.bass as bass
import concourse.mybir as mybir
import concourse.tile as tile
from concourse import bacc
from concourse.bass_utils import run_bass_kernel_spmd
from concourse.masks import make_identity

F32 = mybir.dt.float32
BF16 = mybir.dt.bfloat16
AF = mybir.ActivationFunctionType
ALU = mybir.AluOpType
AX = mybir.AxisListType

B, N, D, H = 4, 2048, 1024, 16
HD = 64
NS = 11
HL = 8
NPAIR = 4
PAD = 1536
NPADCOLS = PAD + N
D4 = [0.4829629131445341, 0.8365163037378079, 0.2241438680420134, -0.1294095225512604]

TAPS = []
for _j in range(NS):
    _d = 1 << _j
    for _tau in range(4):
        _off = _d * _tau
        if _off != 0 and _off >= N:
            continue
        TAPS.append((_j, _tau, _off, 4 * _j + _tau))
NTAP = len(TAPS)            # 42
NTB = 44                    # full (j, tau) grid; invalid slots get coef 0
OFFSETS = sorted({t[2] for t in TAPS})
NOFF = len(OFFSETS)         # 22
OFF_IDX = {o: i for i, o in enumerate(OFFSETS)}
NCH = N // 128
NNB = N // 512

_KERNEL_CACHE = {}


def build_kernel(dbg=False):
    nc = bacc.Bacc("TRN2", target_bir_lowering=False, debug=False, num_devices=8)

    xT = nc.dram_tensor("xT", [D, N], BF16, kind="ExternalInput")
    wT = nc.dram_tensor("wT", [D, 4 * HL * HD], BF16, kind="ExternalInput")
    woT = nc.dram_tensor("woT", [HL * HD, D], BF16, kind="ExternalInput")
    wqs = nc.dram_tensor("wqs", [128, 2 * NS], BF16, kind="ExternalInput")
    sel_c = nc.dram_tensor("sel_c", [128, 2 * NOFF * 128], BF16,
                           kind="ExternalInput")
    prior_c = nc.dram_tensor("prior_c", [128, HL * NS], F32, kind="ExternalInput")
    bias_c = nc.dram_tensor("bias_c", [128, HL * NTB], F32, kind="ExternalInput")
    coef_c = nc.dram_tensor("coef_c", [128, HL * NTB], F32, kind="ExternalInput")
    bqkv_c = nc.dram_tensor("bqkv_c", [128, 12], F32, kind="ExternalInput")
    bgate_c = nc.dram_tensor("bgate_c", [128, NPAIR], F32, kind="ExternalInput")
    bp_c = nc.dram_tensor("bp_c", [128, HL], F32, kind="ExternalInput")

    y_fm = nc.dram_tensor("y_fm", [D, N], F32, kind="ExternalOutput")
    if dbg:
        d_q2 = nc.dram_tensor("d_q2", [128, N], BF16, kind="ExternalOutput")
        d_k2 = nc.dram_tensor("d_k2", [128, NPADCOLS], BF16,
                              kind="ExternalOutput")
        d_v2 = nc.dram_tensor("d_v2", [128, NPADCOLS], BF16,
                              kind="ExternalOutput")
        d_gate = nc.dram_tensor("d_gate", [128, N], BF16, kind="ExternalOutput")
        d_qkT = nc.dram_tensor("d_qkT", [88, N], BF16, kind="ExternalOutput")
        d_E = nc.dram_tensor("d_E", [128, NCH * HL * NS], BF16,
                             kind="ExternalOutput")
        d_qktm = nc.dram_tensor("d_qktm", [128, NCH * 2 * 88], BF16,
                                kind="ExternalOutput")
        d_ast = nc.dram_tensor("d_ast", [88, N], BF16, kind="ExternalOutput")
        d_ofm = nc.dram_tensor("d_ofm", [128, N], BF16, kind="ExternalOutput")

    CH = NCH * HL               # 128 (c,h) groups
    W = NCH * HL * NTB          # 5632

    with tile.TileContext(nc) as tc, ExitStack() as S:
        # ---- persistent pools ----
        const = S.enter_context(tc.tile_pool(name="const", bufs=1))
        big = S.enter_context(tc.tile_pool(name="big", bufs=1, side="right"))
        k2 = [big.tile([128, NPADCOLS], BF16, tag=f"k2_{p}", name=f"k2_{p}")
              for p in range(NPAIR)]
        v2 = [big.tile([128, NPADCOLS], BF16, tag=f"v2_{p}", name=f"v2_{p}")
              for p in range(NPAIR)]
        gate = [big.tile([128, N], BF16, tag=f"g_{p}", name=f"g_{p}")
                for p in range(NPAIR)]
        out_fm = [big.tile([128, N], BF16, tag=f"o_{p}", name=f"o_{p}")
                  for p in range(NPAIR)]

        S_as = ExitStack()
        arow = S_as.enter_context(tc.tile_pool(name="arow", bufs=1, side="right"))
        A_stage = [arow.tile([88, N], BF16, tag=f"ast{t}", name=f"ast{t}")
                   for t in range(2)]

        S_e = ExitStack()
        qk_Tp = S_e.enter_context(tc.tile_pool(name="qkTp", bufs=1, side="right"))
        qk_T = [qk_Tp.tile([88, N], BF16, tag=f"qkT{t}", name=f"qkT{t}")
                for t in range(2)]
        epre = S_e.enter_context(tc.tile_pool(name="epre", bufs=1))
        E_tm = epre.tile([128, NCH * HL * NS], BF16)
        SE_tm = epre.tile([128, NCH * HL], F32)
        EC_t = epre.tile([128, W], BF16)
        qk_tm = epre.tile([128, NCH * 2 * 88], BF16, name="qk_tm")
        wqs_r = const.tile([128, 2 * NS], BF16)
        prior_t = const.tile([128, HL * NS], F32)
        bp_t = const.tile([128, HL], F32)

        # ======== P1: projections (input DMAs first for fast start) ========
        S_q = ExitStack()
        qp = S_q.enter_context(tc.tile_pool(name="qp", bufs=1, side="right"))
        q2 = [qp.tile([128, N], BF16, tag=f"q2_{p}", name=f"q2_{p}")
              for p in range(NPAIR)]
        bqkv_t = const.tile([128, 12], F32)
        bgate_t = const.tile([128, NPAIR], F32)
        ident_f = const.tile([128, 128], F32)
        ident_b = const.tile([128, 128], BF16)
        ones_f = const.tile([128, 2], F32)
        ones2 = const.tile([128, 2], BF16)
        bias_t = const.tile([128, HL * NTB], F32)
        coef_t = const.tile([128, HL * NTB], F32)
        def p2_unit(c, psp, scp):
            ps = psp.tile([128, HL * NS], F32, tag="qs")
            for p in range(NPAIR):
                nc.tensor.matmul(
                    ps[:, 2 * NS * p:2 * NS * (p + 1)],
                    q2[p][:, 128 * c:128 * (c + 1)], wqs_r[:],
                    start=True, stop=True)
            et = scp.tile([128, HL * NS], F32, tag="et")
            nc.vector.tensor_tensor(out=et[:], in0=ps[:], in1=prior_t[:],
                                    op=ALU.add)
            nc.scalar.activation(E_tm[:, HL * NS * c:HL * NS * (c + 1)],
                                 et[:], AF.Exp)

        def p4_unit(ct, psp):
            c, t = ct // 2, ct % 2
            ps = psp.tile([128, 88], BF16, tag="tp")
            nc.tensor.transpose(
                ps[:], qk_T[t][:, 128 * c:128 * (c + 1)],
                ident_b[0:88, 0:88])
            nc.scalar.copy(
                qk_tm[:, 176 * c + 88 * t:176 * c + 88 * (t + 1)], ps[:])

        def p3_unit(oi, off, prp, stp, psp):
            prods = []
            oc = off if off >= 128 else 0   # zero-region worth skipping
            for p in range(NPAIR):
                prod = prp.tile([128, N], BF16, tag=f"prod{p}")
                if oc:
                    nc.gpsimd.memset(prod[:, 0:oc].bitcast(F32), 0.0)
                nc.vector.tensor_tensor(
                    out=prod[:, oc:N], in0=q2[p][:, oc:N],
                    in1=k2[p][:, PAD - off + oc:PAD - off + N], op=ALU.mult)
                prods.append(prod)
            ps = psp.tile([128, N], F32, tag="qkps")
            for nb in range(NNB):
                for p in range(NPAIR):
                    nc.tensor.matmul(
                        ps[32 * p:32 * p + 2, 512 * nb:512 * (nb + 1)],
                        ones2[:], prods[p][:, 512 * nb:512 * (nb + 1)],
                        start=True, stop=True,
                        tile_position=(0, 32 * p))
            st = stp.tile([128, N], BF16, tag="stage")
            nc.scalar.copy(st[:], ps[:])
            for t in range(2):
                for ph in range(2):
                    p = 2 * t + ph
                    dst = qk_T[t][:].rearrange(
                        "(a j o) n -> a j o n", a=2, j=2)[ph, :, oi, :]
                    nc.sync.dma_start(dst, st[32 * p:32 * p + 2, :])

        with tc.tile_pool(name="p1x", bufs=1) as p1x, \
             tc.tile_pool(name="p1", bufs=2) as p1, \
             tc.tile_pool(name="p3a", bufs=1) as p3a, \
             tc.tile_pool(name="p3sa", bufs=2) as p3sa, \
             tc.tile_pool(name="p2sc", bufs=2) as p2sc, \
             tc.tile_pool(name="p3psa", bufs=1, space="PSUM") as p3psa, \
             tc.tile_pool(name="p2psa", bufs=1, space="PSUM") as p2psa, \
             tc.tile_pool(name="p1ps", bufs=3, space="PSUM") as p1ps:
            def load_wr(fc):
                wrt = p1.tile([128, 8 * 128], BF16, tag="wr")
                nc.sync.dma_start(
                    wrt[:].rearrange("p (a m) -> p a m", a=8),
                    wT[:, 128 * fc:128 * (fc + 1)]
                    .rearrange("(a p) m -> p a m", p=128))
                return wrt

            wr_pre = [load_wr(0)]
            nc.sync.dma_start(bqkv_t[:], bqkv_c[:])
            nc.sync.dma_start(bgate_t[:], bgate_c[:])
            nc.sync.dma_start(bias_t[:], bias_c[:])
            nc.sync.dma_start(coef_t[:], coef_c[:])
            nc.sync.dma_start(wqs_r[:], wqs[:])
            nc.sync.dma_start(prior_t[:], prior_c[:])
            nc.sync.dma_start(bp_t[:], bp_c[:])
            make_identity(nc, ident_f)
            nc.vector.tensor_copy(ident_b[:], ident_f[:])
            nc.vector.memset(ones_f[:], 0.0)
            nc.vector.memset(ones_f[0:64, 0:1], 1.0)
            nc.vector.memset(ones_f[64:128, 1:2], 1.0)
            nc.vector.tensor_copy(ones2[:], ones_f[:])
            xr = []
            for kc in range(D // 128):
                xrt = p1x.tile([128, N], BF16, tag=f"xr{kc}", name=f"xr{kc}")
                nc.sync.dma_start(xrt[:], xT[128 * kc:128 * (kc + 1), :])
                xr.append(xrt)
            for p in range(NPAIR):
                nc.vector.memset(k2[p][:, 0:PAD], 0.0)
                nc.vector.memset(v2[p][:, 0:PAD], 0.0)
            for fc in range(16):
                wr = wr_pre[fc] if fc < len(wr_pre) else load_wr(fc)
                sect, pair = fc // 4, fc % 4
                for nb in range(NNB):
                    ps = p1ps.tile([128, 512], F32, tag="proj")
                    for kc in range(D // 128):
                        nc.tensor.matmul(
                            ps[:], wr[:, 128 * kc:128 * (kc + 1)],
                            xr[kc][:, 512 * nb:512 * (nb + 1)],
                            start=(kc == 0), stop=(kc == 7))
                    sl = slice(512 * nb, 512 * (nb + 1))
                    slp = slice(PAD + 512 * nb, PAD + 512 * (nb + 1))
                    if sect == 0:
                        nc.scalar.activation(q2[pair][:, sl], ps[:],
                                             AF.Identity, bias=bqkv_t[:, fc:fc + 1])
                    elif sect == 1:
                        nc.scalar.activation(k2[pair][:, slp], ps[:],
                                             AF.Identity, bias=bqkv_t[:, fc:fc + 1])
                    elif sect == 2:
                        nc.scalar.activation(v2[pair][:, slp], ps[:],
                                             AF.Identity, bias=bqkv_t[:, fc:fc + 1])
                    else:
                        nc.scalar.activation(gate[pair][:, sl], ps[:], AF.Sigmoid,
                                             bias=bgate_t[:, pair:pair + 1])
                if 4 <= fc < 8:
                    for u in range(4):
                        p2_unit(4 * (fc - 4) + u, p2psa, p2sc)
                elif fc >= 8:
                    sched = (3 * (fc - 8) if fc < 14
                             else 18 + 2 * (fc - 14))
                    cnt = 3 if fc < 14 else 2
                    for u in range(cnt):
                        uoi = sched + u
                        p3_unit(uoi, OFFSETS[uoi], p3a, p3sa, p3psa)

        # ---- deferred allocations (land in space freed by P1 transients) ----
        ep = S_e.enter_context(tc.tile_pool(name="ep", bufs=1))
        sel_t = ep.tile([128, 2 * NOFF * 128], BF16)
        nc.sync.dma_start(sel_t[:], sel_c[:])

        nc.vector.tensor_reduce(
            SE_tm[:].rearrange("p (c h) -> p c h", h=HL).unsqueeze(-1),
            E_tm[:].rearrange("p (c h s) -> p c h s", h=HL, s=NS),
            axis=AX.X, op=ALU.add)


        if dbg:
            nc.sync.dma_start(d_q2[:], q2[0][:])
            nc.sync.dma_start(d_k2[:], k2[0][:])
            nc.sync.dma_start(d_v2[:], v2[0][:])
            nc.sync.dma_start(d_gate[:], gate[0][:])

        S_q.close()  # q2 released
        if dbg:
            nc.sync.dma_start(d_qkT[:], qk_T[0][:])
            nc.sync.dma_start(d_E[:], E_tm[:])

        if dbg:
            nc.sync.dma_start(d_qktm[:], qk_tm[:])

        # ======== P4 standalone ========
        with tc.tile_pool(name="p4ps", bufs=4, space="PSUM") as p4ps:
            for ct in range(2 * NCH):
                p4_unit(ct, p4ps)

        # ======== P5..P8 pipelined by token halves / nb blocks ========
        with tc.tile_pool(name="p5", bufs=2) as p5, \
             tc.tile_pool(name="p6ps", bufs=1, space="PSUM") as p6ps, \
             tc.tile_pool(name="p7", bufs=8) as p7, \
             tc.tile_pool(name="p7ps", bufs=4, space="PSUM") as p7ps, \
             tc.tile_pool(name="p7po", bufs=2, space="PSUM") as p7po, \
             tc.tile_pool(name="p8", bufs=2) as p8, \
             tc.tile_pool(name="p8g", bufs=1) as p8g, \
             tc.tile_pool(name="p8ps", bufs=1, space="PSUM") as p8ps:
            wo_r = []
            for p in range(NPAIR):
                wor = p8g.tile([128, D], BF16, tag=f"wor{p}", name=f"wor{p}")
                nc.sync.dma_start(wor[:], woT[128 * p:128 * (p + 1), :])
                wo_r.append(wor)
            sel_v = sel_t[:].rearrange("p (i m) -> p i m", i=2 * NOFF)

            def p5_half(hf):
                c0, c1 = 4 * hf, 4 * hf + 4   # chunk range (one nb block)
                geng = nc.gpsimd
                ncc = c1 - c0                 # 8 chunks
                Wh = ncc * HL * NTB           # 2816
                CHh = ncc * HL
                qk_s = qk_tm[:, 176 * c0:176 * c1]
                qk8 = qk_s.rearrange("p (ch o) -> p ch o", o=NOFF)
                qk9 = qk_s.rearrange("p (ch e two) -> p ch e two", e=NS, two=2)
                EC_s = EC_t[:, HL * NTB * c0:HL * NTB * c1]
                SE_s = SE_tm[:, HL * c0:HL * c1]

                y_t = p5.tile([128, Wh], BF16, tag="y")
                y7 = y_t[:].rearrange("p (ch j f) -> p ch j f", j=NS, f=4)
                geng.tensor_copy(
                    y7[:, :, :, 0:1],
                    qk8[:, :, 0:1].unsqueeze(2).broadcast_to([128, CHh, NS, 1]))
                geng.tensor_copy(y7[:, :, 0:1, 1:2],
                                      qk8[:, :, 1:2].unsqueeze(2))
                geng.tensor_copy(y7[:, :, 1:2, 1:2],
                                      qk8[:, :, 2:3].unsqueeze(2))
                geng.tensor_copy(y7[:, :, 2:11, 1:2],
                                      qk9[:, :, 2:11, 0:1])
                geng.tensor_copy(y7[:, :, 0:1, 2:3],
                                      qk8[:, :, 2:3].unsqueeze(2))
                geng.tensor_copy(y7[:, :, 1:10, 2:3],
                                      qk9[:, :, 2:11, 0:1])
                geng.tensor_copy(y7[:, :, 10:11, 2:3],
                                      qk9[:, :, 10:11, 0:1])
                geng.tensor_copy(y7[:, :, 0:10, 3:4],
                                      qk9[:, :, 1:11, 1:2])
                geng.tensor_copy(y7[:, :, 10:11, 3:4],
                                      qk9[:, :, 10:11, 1:2])
                nc.gpsimd.tensor_copy(
                    EC_s.rearrange("p (ch s f) -> p ch s f", s=NS, f=4),
                    E_tm[:, HL * NS * c0:HL * NS * c1]
                    .rearrange("p (ch s) -> p ch s", s=NS)
                    .unsqueeze(-1).broadcast_to([128, CHh, NS, 4]))
                nc.vector.tensor_tensor(
                    out=EC_s.rearrange("p (c w) -> p c w", w=HL * NTB),
                    in0=EC_s.rearrange("p (c w) -> p c w", w=HL * NTB),
                    in1=coef_t[:].unsqueeze(1)
                    .broadcast_to([128, ncc, HL * NTB]),
                    op=ALU.mult)

                nc.vector.tensor_tensor(
                    out=y_t[:].rearrange("p (c w) -> p c w", w=HL * NTB),
                    in0=y_t[:].rearrange("p (c w) -> p c w", w=HL * NTB),
                    in1=bias_t[:].unsqueeze(1).broadcast_to([128, ncc, HL * NTB]),
                    op=ALU.add)
                m0 = p5.tile([128, Wh], BF16, tag="m0")
                nc.vector.tensor_scalar(out=m0[:], in0=y_t[:], scalar1=0.0,
                                        scalar2=None, op0=ALU.min)
                nc.scalar.activation(m0[:], m0[:], AF.Exp)
                nc.vector.tensor_scalar(out=y_t[:], in0=y_t[:], scalar1=0.0,
                                        scalar2=None, op0=ALU.max)
                nc.vector.tensor_tensor(out=m0[:], in0=m0[:], in1=y_t[:],
                                        op=ALU.add)
                nc.vector.tensor_tensor(out=m0[:], in0=m0[:], in1=EC_s,
                                        op=ALU.mult)

                ab = p5.tile([128, CHh], F32, tag="ab")
                t0 = p5.tile([128, CHh], F32, tag="t0")
                nc.vector.tensor_scalar(out=ab[:], in0=qk8[:, :, 0:1].squeeze(-1),
                                        scalar1=0.0, scalar2=None, op0=ALU.min)
                nc.scalar.activation(ab[:], ab[:], AF.Exp)
                nc.vector.tensor_scalar(out=t0[:], in0=qk8[:, :, 0:1].squeeze(-1),
                                        scalar1=0.0, scalar2=None, op0=ALU.max)
                nc.vector.tensor_tensor(out=ab[:], in0=ab[:], in1=t0[:],
                                        op=ALU.add)
                nc.vector.tensor_tensor(out=ab[:], in0=ab[:], in1=SE_s,
                                        op=ALU.mult)
                nc.vector.tensor_tensor(
                    out=ab[:].rearrange("p (c h) -> p c h", h=HL),
                    in0=ab[:].rearrange("p (c h) -> p c h", h=HL),
                    in1=bp_t[:].unsqueeze(1).broadcast_to([128, ncc, HL]),
                    op=ALU.mult)

                z_t = p5.tile([128, CHh], F32, tag="z")
                nc.vector.tensor_reduce(
                    z_t[:].rearrange("p (c h) -> p c h", h=HL).unsqueeze(-1),
                    m0[:].rearrange("p (c h t) -> p c h t", h=HL, t=NTB),
                    axis=AX.X, op=ALU.add, apply_absolute_value=True)
                nc.vector.tensor_tensor(out=z_t[:], in0=z_t[:], in1=ab[:],
                                        op=ALU.add)
                nc.vector.scalar_tensor_tensor(
                    out=z_t[:], in0=SE_s, scalar=1e-6, in1=z_t[:],
                    op0=ALU.mult, op1=ALU.add)
                nc.vector.reciprocal(z_t[:], z_t[:])
                nc.vector.tensor_tensor(
                    out=m0[:].rearrange("p (ch t) -> p ch t", t=NTB),
                    in0=m0[:].rearrange("p (ch t) -> p ch t", t=NTB),
                    in1=z_t[:].unsqueeze(-1).broadcast_to([128, CHh, NTB]),
                    op=ALU.mult)
                nc.vector.tensor_tensor(out=ab[:], in0=ab[:], in1=z_t[:],
                                        op=ALU.mult)

                A_tm = p5.tile([128, ncc * HL * NOFF], F32, tag="atm")
                A8 = A_tm[:].rearrange("p (ch o) -> p ch o", o=NOFF)
                A9 = A_tm[:].rearrange("p (ch e two) -> p ch e two", e=NS, two=2)
                m7 = m0[:].rearrange("p (ch j f) -> p ch j f", j=NS, f=4)
                nc.vector.tensor_reduce(A8[:, :, 0:1].unsqueeze(-1),
                                        m7[:, :, :, 0:1].transpose([0, 1, 3, 2]),
                                        axis=AX.X, op=ALU.add)
                nc.vector.tensor_tensor(out=A8[:, :, 0:1].squeeze(-1),
                                        in0=A8[:, :, 0:1].squeeze(-1),
                                        in1=ab[:], op=ALU.add)
                nc.vector.tensor_copy(A8[:, :, 1:2],
                                      m7[:, :, 0:1, 1:2].squeeze(-1))
                nc.vector.tensor_tensor(out=A9[:, :, 1:11, 0:1],
                                        in0=m7[:, :, 1:11, 1:2],
                                        in1=m7[:, :, 0:10, 2:3], op=ALU.add)
                nc.vector.tensor_copy(A9[:, :, 1:11, 1:2],
                                      m7[:, :, 0:10, 3:4])

                # P6: transpose to A_stage columns of this half
                for ci in range(ncc):
                    c = c0 + ci
                    for t in range(2):
                        ps2 = p6ps.tile([88, 128], F32, tag="tpb")
                        nc.tensor.transpose(
                            ps2[:],
                            A_tm[:, 176 * ci + 88 * t:176 * ci + 88 * (t + 1)],
                            ident_f[:])
                        nc.scalar.copy(A_stage[t][:, 128 * c:128 * (c + 1)],
                                       ps2[:])

            def p7_block(nb):
                n0 = 512 * nb
                for p in range(NPAIR):
                    t, ph = p // 2, p % 2
                    po = p7po.tile([128, 512], F32, tag="avo")
                    valid = [(oi, off) for oi, off in enumerate(OFFSETS)
                             if off < n0 + 512]
                    for vi, (oi, off) in enumerate(valid):
                        pa = p7ps.tile([128, 512], F32, tag="aexp")
                        nc.tensor.matmul(
                            pa[:], sel_v[0:88, NOFF * ph + oi, :],
                            A_stage[t][:, n0:n0 + 512],
                            start=True, stop=True)
                        tmp = p7.tile([128, 512], BF16, tag="avt")
                        nc.vector.tensor_tensor(
                            out=tmp[:],
                            in0=v2[p][:, PAD + n0 - off:PAD + n0 + 512 - off],
                            in1=pa[:], op=ALU.mult)
                        nc.tensor.matmul(
                            po[:], ident_b[:], tmp[:],
                            start=(vi == 0), stop=(vi == len(valid) - 1))
                    nc.vector.tensor_tensor(
                        out=out_fm[p][:, n0:n0 + 512],
                        in0=gate[p][:, n0:n0 + 512], in1=po[:], op=ALU.mult)

            def p8_block(nb):
                n0 = 512 * nb
                for dc in range(D // 128):
                    ps = p8ps.tile([128, 512], F32, tag="yps")
                    for p in range(NPAIR):
                        nc.tensor.matmul(
                            ps[:], wo_r[p][:, 128 * dc:128 * (dc + 1)],
                            out_fm[p][:, n0:n0 + 512],
                            start=(p == 0), stop=(p == NPAIR - 1))
                    yt = p8.tile([128, 512], F32, tag="yt")
                    nc.scalar.copy(yt[:], ps[:])
                    nc.sync.dma_start(
                        y_fm[128 * dc:128 * (dc + 1), n0:n0 + 512], yt[:])

            p5_half(0)
            p7_block(0)
            p5_half(1)
            p8_block(0)
            p7_block(1)
            p5_half(2)
            p8_block(1)
            p7_block(2)
            p5_half(3)
            if dbg:
                nc.sync.dma_start(d_ast[:], A_stage[0][:])
            p8_block(2)
            p7_block(3)
            p8_block(3)
            if dbg:
                nc.sync.dma_start(d_ofm[:], out_fm[0][:])

        S_e.close()
        S_as.close()
    nc.compile()
    return nc


# ===========================================================================
# host side
# ===========================================================================

_SEL = np.zeros((128, 2 * NOFF * 128), np.float32)
for _ph in range(2):
    for _oi in range(NOFF):
        _i = NOFF * _ph + _oi
        _SEL[44 * _ph + _oi, 128 * _i:128 * _i + 64] = 1.0
        _SEL[44 * _ph + NOFF + _oi, 128 * _i + 64:128 * (_i + 1)] = 1.0


def _bf16(a):
    return np.ascontiguousarray(a).astype(ml_dtypes.bfloat16)


def _make_inputs(x, W_qkv, b_qkv, W_out, W_gate, b_gate, scale_gain, W_qscale,
                 identity_bypass, pos_bias, b, g):
    hg0 = g * HL
    rows = slice(hg0 * HD, (hg0 + HL) * HD)
    Wq = W_qkv[0 * D:1 * D][rows]
    Wk = W_qkv[1 * D:2 * D][rows]
    Wv = W_qkv[2 * D:3 * D][rows]
    Wg = W_gate[rows]
    wTv = np.concatenate([Wq, Wk, Wv, Wg], axis=0).T.copy()
    woTv = W_out[:, rows].T.copy()

    wqsv = np.zeros((128, 2 * NS), np.float32)
    wqsv[0:64, 0:NS] = W_qscale.T
    wqsv[64:128, NS:2 * NS] = W_qscale.T

    prior = np.zeros((HL, NS), np.float32)
    for h in range(HL):
        prior[h] = scale_gain[:, hg0 + h]
    prior_v = np.broadcast_to(prior.reshape(1, -1), (128, HL * NS)).copy()

    bias = np.zeros((HL, NTB), np.float32)
    coef = np.zeros((HL, NTB), np.float32)
    for h in range(HL):
        for (j, tau, off, full_idx) in TAPS:
            bias[h, full_idx] = pos_bias[full_idx, hg0 + h]
            coef[h, full_idx] = D4[tau]
    bias_v = np.broadcast_to(bias.reshape(1, -1), (128, HL * NTB)).copy()
    coef_v = np.broadcast_to(coef.reshape(1, -1), (128, HL * NTB)).copy()

    bqkv = np.zeros((128, 12), np.float32)
    for sect, bb in enumerate([b_qkv[0:D], b_qkv[D:2 * D], b_qkv[2 * D:3 * D]]):
        sl = bb[rows]
        for pair in range(NPAIR):
            bqkv[:, sect * 4 + pair] = sl[128 * pair:128 * (pair + 1)]
    bgate_v = np.zeros((128, NPAIR), np.float32)
    gsl = b_gate[rows]
    for pair in range(NPAIR):
        bgate_v[:, pair] = gsl[128 * pair:128 * (pair + 1)]

    bp = np.log1p(np.exp(identity_bypass[hg0:hg0 + HL])).astype(np.float32)
    bp_v = np.broadcast_to(bp.reshape(1, -1), (128, HL)).copy()

    return {
        "xT": _bf16(x[b].T),
        "wT": _bf16(wTv),
        "woT": _bf16(woTv),
        "wqs": _bf16(wqsv),
        "sel_c": _bf16(_SEL),
        "prior_c": np.ascontiguousarray(prior_v),
        "bias_c": np.ascontiguousarray(bias_v),
        "coef_c": np.ascontiguousarray(coef_v),
        "bqkv_c": bqkv,
        "bgate_c": bgate_v,
        "bp_c": np.ascontiguousarray(bp_v),
    }


def kernel(x, W_qkv, b_qkv, W_out, b_out, W_gate, b_gate, scale_gain, W_qscale,
           identity_bypass, pos_bias):
    x = np.asarray(x, np.float32)
    args = [np.asarray(a, np.float32) for a in
            (W_qkv, b_qkv, W_out, W_gate, b_gate, scale_gain, W_qscale,
             identity_bypass, pos_bias)]
    (W_qkv, b_qkv, W_out, W_gate, b_gate, scale_gain, W_qscale,
     identity_bypass, pos_bias) = args

    if "nc" not in _KERNEL_CACHE:
        _KERNEL_CACHE["nc"] = build_kernel()
    nc = _KERNEL_CACHE["nc"]

    in_maps = []
    for core in range(8):
        b, g = core % 4, core // 4
        in_maps.append(_make_inputs(x, W_qkv, b_qkv, W_out, W_gate, b_gate,
                                    scale_gain, W_qscale, identity_bypass,
                                    pos_bias, b, g))
    res = run_bass_kernel_spmd(nc, in_maps, list(range(8)))

    out = np.zeros((B, N, D), np.float32)
    for core in range(8):
        b = core % 4
        out[b] += res.results[core]["y_fm"].T
    out += np.asarray(b_out, np.float32)
    return out


# revision 36
# speedup vs baseline: 1.1872x; 1.1872x over previous
"""DWARF attention Trainium2 Bass kernel (v3, bf16 + pipelined halves).

Sharding: 8 cores = 4 batches x 2 head-halves (8 local heads each).
Per-core dataflow (feature-major = [feature rows, token cols]):
  P1 proj:  q/k/v/gate = W^T.T @ xT on PE (bf16), ACT evictions w/ bias+sigmoid
            k/v evicted into left-zero-padded resident tiles (shifted reads)
  P2 E:     E = exp(q_offset + prior) token-major (bf16), SE row-sums (f32),
            EC = E*coef tap table built off critical path
  P3 qk:    per offset: 4 pair-products (DVE bf16) -> 16 pair-sum matmuls into
            one 4-bank psum tile at row bases {0,32,64,96} -> 4 direct
            psum->SBUF row DMAs into qk_T [88,N] f32
  P4 tm:    PE-transpose qk_T -> qk_tm token-major bf16
  P5 feat:  (per token-half) tap-gathers, feat=elu(qk+b)+1, A=EC*feat/z
  P6 A_T:   (per half) PE-transpose A_tm (f32) -> A_stage [88,N] bf16
  P7 AV:    (per nb) per (pair,off): sel-matmul expand, DVE mul w/ padded v2,
            PE identity accumulate
  P8 out:   (per nb) gg = out_fm*gate; y_fm = Wout^T.T @ gg (PE bf16) -> DRAM
Host: shard, pre-transpose weights to bf16, build sel/tap tables, reduce
head-halves.
"""
from contextlib import ExitStack

import ml_dtypes
import numpy as np

import concourse.bass as bass
import concourse.mybir as mybir
import concourse.tile as tile
from concourse import bacc
from concourse.bass_utils import run_bass_kernel_spmd
from concourse.masks import make_identity

F32 = mybir.dt.float32
BF16 = mybir.dt.bfloat16
AF = mybir.ActivationFunctionType
ALU = mybir.AluOpType
AX = mybir.AxisListType

B, N, D, H = 4, 2048, 1024, 16
HD = 64
NS = 11
HL = 8
NPAIR = 4
PAD = 1536
NPADCOLS = PAD + N
D4 = [0.4829629131445341, 0.8365163037378079, 0.2241438680420134, -0.1294095225512604]

TAPS = []
for _j in range(NS):
    _d = 1 << _j
    for _tau in range(4):
        _off = _d * _tau
        if _off != 0 and _off >= N:
            continue
        TAPS.append((_j, _tau, _off, 4 * _j + _tau))
NTAP = len(TAPS)            # 42
NTB = 44                    # full (j, tau) grid; invalid slots get coef 0
OFFSETS = sorted({t[2] for t in TAPS})
NOFF = len(OFFSETS)         # 22
OFF_IDX = {o: i for i, o in enumerate(OFFSETS)}
NCH = N // 128
NNB = N // 512

_KERNEL_CACHE = {}


def build_kernel(dbg=False):
    nc = bacc.Bacc("TRN2", target_bir_lowering=False, debug=False, num_devices=8)

    xT = nc.dram_tensor("xT", [D, N], BF16, kind="ExternalInput")
    wT = nc.dram_tensor("wT", [D, 4 * HL * HD], BF16, kind="ExternalInput")
    woT = nc.dram_tensor("woT", [HL * HD, D], BF16, kind="ExternalInput")
    wqs = nc.dram_tensor("wqs", [128, 2 * NS], BF16, kind="ExternalInput")
    sel_c = nc.dram_tensor("sel_c", [128, 2 * NOFF * 128], BF16,
                           kind="ExternalInput")
    prior_c = nc.dram_tensor("prior_c", [128, HL * NS], F32, kind="ExternalInput")
    bias_c = nc.dram_tensor("bias_c", [128, HL * NTB], F32, kind="ExternalInput")
    coef_c = nc.dram_tensor("coef_c", [128, HL * NTB], F32, kind="ExternalInput")
    bqkv_c = nc.dram_tensor("bqkv_c", [128, 12], F32, kind="ExternalInput")
    bgate_c = nc.dram_tensor("bgate_c", [128, NPAIR], F32, kind="ExternalInput")
    bp_c = nc.dram_tensor("bp_c", [128, HL], F32, kind="ExternalInput")

    y_fm = nc.dram_tensor("y_fm", [D, N], F32, kind="ExternalOutput")
    if dbg:
        d_q2 = nc.dram_tensor("d_q2", [128, N], BF16, kind="ExternalOutput")
        d_k2 = nc.dram_tensor("d_k2", [128, NPADCOLS], BF16,
                              kind="ExternalOutput")
        d_v2 = nc.dram_tensor("d_v2", [128, NPADCOLS], BF16,
                              kind="ExternalOutput")
        d_gate = nc.dram_tensor("d_gate", [128, N], BF16, kind="ExternalOutput")
        d_qkT = nc.dram_tensor("d_qkT", [88, N], BF16, kind="ExternalOutput")
        d_E = nc.dram_tensor("d_E", [128, NCH * HL * NS], BF16,
                             kind="ExternalOutput")
        d_qktm = nc.dram_tensor("d_qktm", [128, NCH * 2 * 88], BF16,
                                kind="ExternalOutput")
        d_ast = nc.dram_tensor("d_ast", [88, N], BF16, kind="ExternalOutput")
        d_ofm = nc.dram_tensor("d_ofm", [128, N], BF16, kind="ExternalOutput")

    CH = NCH * HL               # 128 (c,h) groups
    W = NCH * HL * NTB          # 5632

    with tile.TileContext(nc) as tc, ExitStack() as S:
        # ---- persistent pools ----
        const = S.enter_context(tc.tile_pool(name="const", bufs=1))
        big = S.enter_context(tc.tile_pool(name="big", bufs=1, side="right"))
        k2 = [big.tile([128, NPADCOLS], BF16, tag=f"k2_{p}", name=f"k2_{p}")
              for p in range(NPAIR)]
        v2 = [big.tile([128, NPADCOLS], BF16, tag=f"v2_{p}", name=f"v2_{p}")
              for p in range(NPAIR)]
        gate = [big.tile([128, N], BF16, tag=f"g_{p}", name=f"g_{p}")
                for p in range(NPAIR)]
        out_fm = [big.tile([128, N], BF16, tag=f"o_{p}", name=f"o_{p}")
                  for p in range(NPAIR)]

        S_as = ExitStack()
        arow = S_as.enter_context(tc.tile_pool(name="arow", bufs=1, side="right"))
        A_stage = [arow.tile([88, N], BF16, tag=f"ast{t}", name=f"ast{t}")
                   for t in range(2)]

        S_e = ExitStack()
        qk_Tp = S_e.enter_context(tc.tile_pool(name="qkTp", bufs=1, side="right"))
        qk_T = [qk_Tp.tile([88, N], BF16, tag=f"qkT{t}", name=f"qkT{t}")
                for t in range(2)]
        epre = S_e.enter_context(tc.tile_pool(name="epre", bufs=1))
        E_tm = epre.tile([128, NCH * HL * NS], BF16)
        SE_tm = epre.tile([128, NCH * HL], F32)
        EC_t = epre.tile([128, W], BF16)
        qk_tm = epre.tile([128, NCH * 2 * 88], BF16, name="qk_tm")
        wqs_r = const.tile([128, 2 * NS], BF16)
        prior_t = const.tile([128, HL * NS], F32)
        bp_t = const.tile([128, HL], F32)

        # ======== P1: projections (input DMAs first for fast start) ========
        S_q = ExitStack()
        qp = S_q.enter_context(tc.tile_pool(name="qp", bufs=1, side="right"))
        q2 = [qp.tile([128, N], BF16, tag=f"q2_{p}", name=f"q2_{p}")
              for p in range(NPAIR)]
        bqkv_t = const.tile([128, 12], F32)
        bgate_t = const.tile([128, NPAIR], F32)
        ident_f = const.tile([128, 128], F32)
        ident_b = const.tile([128, 128], BF16)
        ones_f = const.tile([128, 2], F32)
        ones2 = const.tile([128, 2], BF16)
        bias_t = const.tile([128, HL * NTB], F32)
        coef_t = const.tile([128, HL * NTB], F32)
        def p2_unit(c, psp, scp):
            ps = psp.tile([128, HL * NS], F32, tag="qs")
            for p in range(NPAIR):
                nc.tensor.matmul(
                    ps[:, 2 * NS * p:2 * NS * (p + 1)],
                    q2[p][:, 128 * c:128 * (c + 1)], wqs_r[:],
                    start=True, stop=True)
            et = scp.tile([128, HL * NS], F32, tag="et")
            nc.vector.tensor_tensor(out=et[:], in0=ps[:], in1=prior_t[:],
                                    op=ALU.add)
            nc.scalar.activation(E_tm[:, HL * NS * c:HL * NS * (c + 1)],
                                 et[:], AF.Exp)

        def p4_unit(ct, psp):
            c, t = ct // 2, ct % 2
            ps = psp.tile([128, 88], BF16, tag="tp")
            nc.tensor.transpose(
                ps[:], qk_T[t][:, 128 * c:128 * (c + 1)],
                ident_b[0:88, 0:88])
            nc.scalar.copy(
                qk_tm[:, 176 * c + 88 * t:176 * c + 88 * (t + 1)], ps[:])

        def p3_unit(oi, off, prp, stp, psp):
            prods = []
            oc = off if off >= 128 else 0   # zero-region worth skipping
            for p in range(NPAIR):
                prod = prp.tile([128, N], BF16, tag=f"prod{p}")
                if oc:
                    nc.gpsimd.memset(prod[:, 0:oc].bitcast(F32), 0.0)
                nc.vector.tensor_tensor(
                    out=prod[:, oc:N], in0=q2[p][:, oc:N],
                    in1=k2[p][:, PAD - off + oc:PAD - off + N], op=ALU.mult)
                prods.append(prod)
            ps = psp.tile([128, N], F32, tag="qkps")
            for nb in range(NNB):
                for p in range(NPAIR):
                    nc.tensor.matmul(
                        ps[32 * p:32 * p + 2, 512 * nb:512 * (nb + 1)],
                        ones2[:], prods[p][:, 512 * nb:512 * (nb + 1)],
                        start=True, stop=True,
                        tile_position=(0, 32 * p))
            st = stp.tile([128, N], BF16, tag="stage")
            nc.scalar.copy(st[:], ps[:])
            for t in range(2):
                for ph in range(2):
                    p = 2 * t + ph
                    dst = qk_T[t][:].rearrange(
                        "(a j o) n -> a j o n", a=2, j=2)[ph, :, oi, :]
                    nc.sync.dma_start(dst, st[32 * p:32 * p + 2, :])

        with tc.tile_pool(name="p1x", bufs=1) as p1x, \
             tc.tile_pool(name="p1", bufs=2) as p1, \
             tc.tile_pool(name="p3a", bufs=1) as p3a, \
             tc.tile_pool(name="p3sa", bufs=2) as p3sa, \
             tc.tile_pool(name="p2sc", bufs=2) as p2sc, \
             tc.tile_pool(name="p3psa", bufs=1, space="PSUM") as p3psa, \
             tc.tile_pool(name="p2psa", bufs=1, space="PSUM") as p2psa, \
             tc.tile_pool(name="p1ps", bufs=3, space="PSUM") as p1ps:
            def load_wr(fc):
                wrt = p1.tile([128, 8 * 128], BF16, tag="wr")
                nc.sync.dma_start(
                    wrt[:].rearrange("p (a m) -> p a m", a=8),
                    wT[:, 128 * fc:128 * (fc + 1)]
                    .rearrange("(a p) m -> p a m", p=128))
                return wrt

            wr_pre = [load_wr(0)]
            nc.sync.dma_start(bqkv_t[:], bqkv_c[:])
            nc.sync.dma_start(bgate_t[:], bgate_c[:])
            nc.sync.dma_start(bias_t[:], bias_c[:])
            nc.sync.dma_start(coef_t[:], coef_c[:])
            nc.sync.dma_start(wqs_r[:], wqs[:])
            nc.sync.dma_start(prior_t[:], prior_c[:])
            nc.sync.dma_start(bp_t[:], bp_c[:])
            make_identity(nc, ident_f)
            nc.vector.tensor_copy(ident_b[:], ident_f[:])
            nc.vector.memset(ones_f[:], 0.0)
            nc.vector.memset(ones_f[0:64, 0:1], 1.0)
            nc.vector.memset(ones_f[64:128, 1:2], 1.0)
            nc.vector.tensor_copy(ones2[:], ones_f[:])
            xr = []
            for kc in range(D // 128):
                xrt = p1x.tile([128, N], BF16, tag=f"xr{kc}", name=f"xr{kc}")
                nc.sync.dma_start(xrt[:], xT[128 * kc:128 * (kc + 1), :])
                xr.append(xrt)
            for p in range(NPAIR):
                nc.vector.memset(k2[p][:, 0:PAD], 0.0)
                nc.vector.memset(v2[p][:, 0:PAD], 0.0)
            for fc in range(16):
                wr = wr_pre[fc] if fc < len(wr_pre) else load_wr(fc)
                sect, pair = fc // 4, fc % 4
                for nb in range(NNB):
                    ps = p1ps.tile([128, 512], F32, tag="proj")
                    for kc in range(D // 128):
                        nc.tensor.matmul(
                            ps[:], wr[:, 128 * kc:128 * (kc + 1)],
                            xr[kc][:, 512 * nb:512 * (nb + 1)],
                            start=(kc == 0), stop=(kc == 7))
                    sl = slice(512 * nb, 512 * (nb + 1))
                    slp = slice(PAD + 512 * nb, PAD + 512 * (nb + 1))
                    if sect == 0:
                        nc.scalar.activation(q2[pair][:, sl], ps[:],
                                             AF.Identity, bias=bqkv_t[:, fc:fc + 1])
                    elif sect == 1:
                        nc.scalar.activation(k2[pair][:, slp], ps[:],
                                             AF.Identity, bias=bqkv_t[:, fc:fc + 1])
                    elif sect == 2:
                        nc.scalar.activation(v2[pair][:, slp], ps[:],
                                             AF.Identity, bias=bqkv_t[:, fc:fc + 1])
                    else:
                        nc.scalar.activation(gate[pair][:, sl], ps[:], AF.Sigmoid,
                                             bias=bgate_t[:, pair:pair + 1])
                if 4 <= fc < 8:
                    for u in range(4):
                        p2_unit(4 * (fc - 4) + u, p2psa, p2sc)
                elif fc >= 8:
                    sched = (3 * (fc - 8) if fc < 14
                             else 18 + 2 * (fc - 14))
                    cnt = 3 if fc < 14 else 2
                    for u in range(cnt):
                        uoi = sched + u
                        p3_unit(uoi, OFFSETS[uoi], p3a, p3sa, p3psa)

        # ---- deferred allocations (land in space freed by P1 transients) ----
        ep = S_e.enter_context(tc.tile_pool(name="ep", bufs=1))
        sel_t = ep.tile([128, 2 * NOFF * 128], BF16)
        nc.sync.dma_start(sel_t[:], sel_c[:])

        nc.vector.tensor_reduce(
            SE_tm[:].rearrange("p (c h) -> p c h", h=HL).unsqueeze(-1),
            E_tm[:].rearrange("p (c h s) -> p c h s", h=HL, s=NS),
            axis=AX.X, op=ALU.add)


        if dbg:
            nc.sync.dma_start(d_q2[:], q2[0][:])
            nc.sync.dma_start(d_k2[:], k2[0][:])
            nc.sync.dma_start(d_v2[:], v2[0][:])
            nc.sync.dma_start(d_gate[:], gate[0][:])

        S_q.close()  # q2 released
        if dbg:
            nc.sync.dma_start(d_qkT[:], qk_T[0][:])
            nc.sync.dma_start(d_E[:], E_tm[:])

        if dbg:
            nc.sync.dma_start(d_qktm[:], qk_tm[:])

        # ======== P4 standalone ========
        with tc.tile_pool(name="p4ps", bufs=4, space="PSUM") as p4ps:
            for ct in range(2 * NCH):
                p4_unit(ct, p4ps)

        # ======== P5..P8 pipelined by token halves / nb blocks ========
        with tc.tile_pool(name="p5", bufs=2) as p5, \
             tc.tile_pool(name="p6ps", bufs=1, space="PSUM") as p6ps, \
             tc.tile_pool(name="p7", bufs=8) as p7, \
             tc.tile_pool(name="p7ps", bufs=4, space="PSUM") as p7ps, \
             tc.tile_pool(name="p7po", bufs=2, space="PSUM") as p7po, \
             tc.tile_pool(name="p8", bufs=2) as p8, \
             tc.tile_pool(name="p8g", bufs=1) as p8g, \
             tc.tile_pool(name="p8ps", bufs=1, space="PSUM") as p8ps:
            wo_r = []
            for p in range(NPAIR):
                wor = p8g.tile([128, D], BF16, tag=f"wor{p}", name=f"wor{p}")
                nc.sync.dma_start(wor[:], woT[128 * p:128 * (p + 1), :])
                wo_r.append(wor)
            sel_v = sel_t[:].rearrange("p (i m) -> p i m", i=2 * NOFF)

            def p5_half(hf):
                c0, c1 = 4 * hf, 4 * hf + 4   # chunk range (one nb block)
                geng = nc.gpsimd
                ncc = c1 - c0                 # 8 chunks
                Wh = ncc * HL * NTB           # 2816
                CHh = ncc * HL
                qk_s = qk_tm[:, 176 * c0:176 * c1]
                qk8 = qk_s.rearrange("p (ch o) -> p ch o", o=NOFF)
                qk9 = qk_s.rearrange("p (ch e two) -> p ch e two", e=NS, two=2)
                EC_s = EC_t[:, HL * NTB * c0:HL * NTB * c1]
                SE_s = SE_tm[:, HL * c0:HL * c1]

                y_t = p5.tile([128, Wh], BF16, tag="y")
                y7 = y_t[:].rearrange("p (ch j f) -> p ch j f", j=NS, f=4)
                geng.tensor_copy(
                    y7[:, :, :, 0:1],
                    qk8[:, :, 0:1].unsqueeze(2).broadcast_to([128, CHh, NS, 1]))
                geng.tensor_copy(y7[:, :, 0:1, 1:2],
                                      qk8[:, :, 1:2].unsqueeze(2))
                geng.tensor_copy(y7[:, :, 1:2, 1:2],
                                      qk8[:, :, 2:3].unsqueeze(2))
                geng.tensor_copy(y7[:, :, 2:11, 1:2],
                                      qk9[:, :, 2:11, 0:1])
                geng.tensor_copy(y7[:, :, 0:1, 2:3],
                                      qk8[:, :, 2:3].unsqueeze(2))
                geng.tensor_copy(y7[:, :, 1:10, 2:3],
                                      qk9[:, :, 2:11, 0:1])
                geng.tensor_copy(y7[:, :, 10:11, 2:3],
                                      qk9[:, :, 10:11, 0:1])
                geng.tensor_copy(y7[:, :, 0:10, 3:4],
                                      qk9[:, :, 1:11, 1:2])
                geng.tensor_copy(y7[:, :, 10:11, 3:4],
                                      qk9[:, :, 10:11, 1:2])
                nc.gpsimd.tensor_copy(
                    EC_s.rearrange("p (ch s f) -> p ch s f", s=NS, f=4),
                    E_tm[:, HL * NS * c0:HL * NS * c1]
                    .rearrange("p (ch s) -> p ch s", s=NS)
                    .unsqueeze(-1).broadcast_to([128, CHh, NS, 4]))
                nc.vector.tensor_tensor(
                    out=EC_s.rearrange("p (c w) -> p c w", w=HL * NTB),
                    in0=EC_s.rearrange("p (c w) -> p c w", w=HL * NTB),
                    in1=coef_t[:].unsqueeze(1)
                    .broadcast_to([128, ncc, HL * NTB]),
                    op=ALU.mult)

                nc.vector.tensor_tensor(
                    out=y_t[:].rearrange("p (c w) -> p c w", w=HL * NTB),
                    in0=y_t[:].rearrange("p (c w) -> p c w", w=HL * NTB),
                    in1=bias_t[:].unsqueeze(1).broadcast_to([128, ncc, HL * NTB]),
                    op=ALU.add)
                m0 = p5.tile([128, Wh], BF16, tag="m0")
                nc.vector.tensor_scalar(out=m0[:], in0=y_t[:], scalar1=0.0,
                                        scalar2=None, op0=ALU.min)
                nc.scalar.activation(m0[:], m0[:], AF.Exp)
                nc.vector.tensor_scalar(out=y_t[:], in0=y_t[:], scalar1=0.0,
                                        scalar2=None, op0=ALU.max)
                nc.vector.tensor_tensor(out=m0[:], in0=m0[:], in1=y_t[:],
                                        op=ALU.add)
                nc.vector.tensor_tensor(out=m0[:], in0=m0[:], in1=EC_s,
                                        op=ALU.mult)

                ab = p5.tile([128, CHh], F32, tag="ab")
                t0 = p5.tile([128, CHh], F32, tag="t0")
                nc.vector.tensor_scalar(out=ab[:], in0=qk8[:, :, 0:1].squeeze(-1),
                                        scalar1=0.0, scalar2=None, op0=ALU.min)
                nc.scalar.activation(ab[:], ab[:], AF.Exp)
                nc.vector.tensor_scalar(out=t0[:], in0=qk8[:, :, 0:1].squeeze(-1),
                                        scalar1=0.0, scalar2=None, op0=ALU.max)
                nc.vector.tensor_tensor(out=ab[:], in0=ab[:], in1=t0[:],
                                        op=ALU.add)
                nc.vector.tensor_tensor(out=ab[:], in0=ab[:], in1=SE_s,
                                        op=ALU.mult)
                nc.vector.tensor_tensor(
                    out=ab[:].rearrange("p (c h) -> p c h", h=HL),
                    in0=ab[:].rearrange("p (c h) -> p c h", h=HL),
                    in1=bp_t[:].unsqueeze(1).broadcast_to([128, ncc, HL]),
                    op=ALU.mult)

                z_t = p5.tile([128, CHh], F32, tag="z")
                nc.vector.tensor_reduce(
                    z_t[:].rearrange("p (c h) -> p c h", h=HL).unsqueeze(-1),
                    m0[:].rearrange("p (c h t) -> p c h t", h=HL, t=NTB),
                    axis=AX.X, op=ALU.add, apply_absolute_value=True)
                nc.vector.tensor_tensor(out=z_t[:], in0=z_t[:], in1=ab[:],
                                        op=ALU.add)
                nc.vector.scalar_tensor_tensor(
                    out=z_t[:], in0=SE_s, scalar=1e-6, in1=z_t[:],
                    op0=ALU.mult, op1=ALU.add)
                nc.vector.reciprocal(z_t[:], z_t[:])
                nc.vector.tensor_tensor(
                    out=m0[:].rearrange("p (ch t) -> p ch t", t=NTB),
                    in0=m0[:].rearrange("p (ch t) -> p ch t", t=NTB),
                    in1=z_t[:].unsqueeze(-1).broadcast_to([128, CHh, NTB]),
                    op=ALU.mult)
                nc.vector.tensor_tensor(out=ab[:], in0=ab[:], in1=z_t[:],
                                        op=ALU.mult)

                A_tm = p5.tile([128, ncc * HL * NOFF], F32, tag="atm")
                A8 = A_tm[:].rearrange("p (ch o) -> p ch o", o=NOFF)
                A9 = A_tm[:].rearrange("p (ch e two) -> p ch e two", e=NS, two=2)
                m7 = m0[:].rearrange("p (ch j f) -> p ch j f", j=NS, f=4)
                nc.vector.tensor_reduce(A8[:, :, 0:1].unsqueeze(-1),
                                        m7[:, :, :, 0:1].transpose([0, 1, 3, 2]),
                                        axis=AX.X, op=ALU.add)
                nc.vector.tensor_tensor(out=A8[:, :, 0:1].squeeze(-1),
                                        in0=A8[:, :, 0:1].squeeze(-1),
                                        in1=ab[:], op=ALU.add)
                nc.vector.tensor_copy(A8[:, :, 1:2],
                                      m7[:, :, 0:1, 1:2].squeeze(-1))
                nc.vector.tensor_tensor(out=A9[:, :, 1:11, 0:1],
                                        in0=m7[:, :, 1:11, 1:2],
                                        in1=m7[:, :, 0:10, 2:3], op=ALU.add)
                nc.vector.tensor_copy(A9[:, :, 1:11, 1:2],
                                      m7[:, :, 0:10, 3:4])

                # P6: transpose to A_stage columns of this half
                for ci in range(ncc):
                    c = c0 + ci
                    for t in range(2):
                        ps2 = p6ps.tile([88, 128], F32, tag="tpb")
                        nc.tensor.transpose(
                            ps2[:],
                            A_tm[:, 176 * ci + 88 * t:176 * ci + 88 * (t + 1)],
                            ident_f[:])
                        nc.scalar.copy(A_stage[t][:, 128 * c:128 * (c + 1)],
                                       ps2[:])

            def p7_block(nb):
                n0 = 512 * nb
                for p in range(NPAIR):
                    t, ph = p // 2, p % 2
                    po = p7po.tile([128, 512], F32, tag="avo")
                    valid = [(oi, off) for oi, off in enumerate(OFFSETS)
                             if off < n0 + 512]
                    for vi, (oi, off) in enumerate(valid):
                        pa = p7ps.tile([128, 512], F32, tag="aexp")
                        nc.tensor.matmul(
                            pa[:], sel_v[0:88, NOFF * ph + oi, :],
                            A_stage[t][:, n0:n0 + 512],
                            start=True, stop=True)
                        tmp = p7.tile([128, 512], BF16, tag="avt")
                        nc.vector.tensor_tensor(
                            out=tmp[:],
                            in0=v2[p][:, PAD + n0 - off:PAD + n0 + 512 - off],
                            in1=pa[:], op=ALU.mult)
                        nc.tensor.matmul(
                            po[:], ident_b[:], tmp[:],
                            start=(vi == 0), stop=(vi == len(valid) - 1))
                    nc.scalar.copy(out_fm[p][:, n0:n0 + 512], po[:])

            def p8_block(nb):
                n0 = 512 * nb
                for p in range(NPAIR):
                    nc.gpsimd.tensor_tensor(
                        out=out_fm[p][:, n0:n0 + 512],
                        in0=out_fm[p][:, n0:n0 + 512],
                        in1=gate[p][:, n0:n0 + 512], op=ALU.mult)
                for dc in range(D // 128):
                    ps = p8ps.tile([128, 512], F32, tag="yps")
                    for p in range(NPAIR):
                        nc.tensor.matmul(
                            ps[:], wo_r[p][:, 128 * dc:128 * (dc + 1)],
                            out_fm[p][:, n0:n0 + 512],
                            start=(p == 0), stop=(p == NPAIR - 1))
                    yt = p8.tile([128, 512], F32, tag="yt")
                    nc.scalar.copy(yt[:], ps[:])
                    nc.sync.dma_start(
                        y_fm[128 * dc:128 * (dc + 1), n0:n0 + 512], yt[:])

            p5_half(0)
            p7_block(0)
            p5_half(1)
            p8_block(0)
            p7_block(1)
            p5_half(2)
            p8_block(1)
            p7_block(2)
            p5_half(3)
            if dbg:
                nc.sync.dma_start(d_ast[:], A_stage[0][:])
            p8_block(2)
            p7_block(3)
            p8_block(3)
            if dbg:
                nc.sync.dma_start(d_ofm[:], out_fm[0][:])

        S_e.close()
        S_as.close()
    nc.compile()
    return nc


# ===========================================================================
# host side
# ===========================================================================

_SEL = np.zeros((128, 2 * NOFF * 128), np.float32)
for _ph in range(2):
    for _oi in range(NOFF):
        _i = NOFF * _ph + _oi
        _SEL[44 * _ph + _oi, 128 * _i:128 * _i + 64] = 1.0
        _SEL[44 * _ph + NOFF + _oi, 128 * _i + 64:128 * (_i + 1)] = 1.0


def _bf16(a):
    return np.ascontiguousarray(a).astype(ml_dtypes.bfloat16)


def _make_inputs(x, W_qkv, b_qkv, W_out, W_gate, b_gate, scale_gain, W_qscale,
                 identity_bypass, pos_bias, b, g):
    hg0 = g * HL
    rows = slice(hg0 * HD, (hg0 + HL) * HD)
    Wq = W_qkv[0 * D:1 * D][rows]
    Wk = W_qkv[1 * D:2 * D][rows]
    Wv = W_qkv[2 * D:3 * D][rows]
    Wg = W_gate[rows]
    wTv = np.concatenate([Wq, Wk, Wv, Wg], axis=0).T.copy()
    woTv = W_out[:, rows].T.copy()

    wqsv = np.zeros((128, 2 * NS), np.float32)
    wqsv[0:64, 0:NS] = W_qscale.T
    wqsv[64:128, NS:2 * NS] = W_qscale.T

    prior = np.zeros((HL, NS), np.float32)
    for h in range(HL):
        prior[h] = scale_gain[:, hg0 + h]
    prior_v = np.broadcast_to(prior.reshape(1, -1), (128, HL * NS)).copy()

    bias = np.zeros((HL, NTB), np.float32)
    coef = np.zeros((HL, NTB), np.float32)
    for h in range(HL):
        for (j, tau, off, full_idx) in TAPS:
            bias[h, full_idx] = pos_bias[full_idx, hg0 + h]
            coef[h, full_idx] = D4[tau]
    bias_v = np.broadcast_to(bias.reshape(1, -1), (128, HL * NTB)).copy()
    coef_v = np.broadcast_to(coef.reshape(1, -1), (128, HL * NTB)).copy()

    bqkv = np.zeros((128, 12), np.float32)
    for sect, bb in enumerate([b_qkv[0:D], b_qkv[D:2 * D], b_qkv[2 * D:3 * D]]):
        sl = bb[rows]
        for pair in range(NPAIR):
            bqkv[:, sect * 4 + pair] = sl[128 * pair:128 * (pair + 1)]
    bgate_v = np.zeros((128, NPAIR), np.float32)
    gsl = b_gate[rows]
    for pair in range(NPAIR):
        bgate_v[:, pair] = gsl[128 * pair:128 * (pair + 1)]

    bp = np.log1p(np.exp(identity_bypass[hg0:hg0 + HL])).astype(np.float32)
    bp_v = np.broadcast_to(bp.reshape(1, -1), (128, HL)).copy()

    return {
        "xT": _bf16(x[b].T),
        "wT": _bf16(wTv),
        "woT": _bf16(woTv),
        "wqs": _bf16(wqsv),
        "sel_c": _bf16(_SEL),
        "prior_c": np.ascontiguousarray(prior_v),
        "bias_c": np.ascontiguousarray(bias_v),
        "coef_c": np.ascontiguousarray(coef_v),
        "bqkv_c": bqkv,
        "bgate_c": bgate_v,
        "bp_c": np.ascontiguousarray(bp_v),
    }


def kernel(x, W_qkv, b_qkv, W_out, b_out, W_gate, b_gate, scale_gain, W_qscale,
           identity_bypass, pos_bias):
    x = np.asarray(x, np.float32)
    args = [np.asarray(a, np.float32) for a in
            (W_qkv, b_qkv, W_out, W_gate, b_gate, scale_gain, W_qscale,
             identity_bypass, pos_bias)]
    (W_qkv, b_qkv, W_out, W_gate, b_gate, scale_gain, W_qscale,
     identity_bypass, pos_bias) = args

    if "nc" not in _KERNEL_CACHE:
        _KERNEL_CACHE["nc"] = build_kernel()
    nc = _KERNEL_CACHE["nc"]

    in_maps = []
    for core in range(8):
        b, g = core % 4, core // 4
        in_maps.append(_make_inputs(x, W_qkv, b_qkv, W_out, W_gate, b_gate,
                                    scale_gain, W_qscale, identity_bypass,
                                    pos_bias, b, g))
    res = run_bass_kernel_spmd(nc, in_maps, list(range(8)))

    out = np.zeros((B, N, D), np.float32)
    for core in range(8):
        b = core % 4
        out[b] += res.results[core]["y_fm"].T
    out += np.asarray(b_out, np.float32)
    return out


# revision 37
# speedup vs baseline: 1.1909x; 1.0032x over previous
"""DWARF attention Trainium2 Bass kernel (v3, bf16 + pipelined halves).

Sharding: 8 cores = 4 batches x 2 head-halves (8 local heads each).
Per-core dataflow (feature-major = [feature rows, token cols]):
  P1 proj:  q/k/v/gate = W^T.T @ xT on PE (bf16), ACT evictions w/ bias+sigmoid
            k/v evicted into left-zero-padded resident tiles (shifted reads)
  P2 E:     E = exp(q_offset + prior) token-major (bf16), SE row-sums (f32),
            EC = E*coef tap table built off critical path
  P3 qk:    per offset: 4 pair-products (DVE bf16) -> 16 pair-sum matmuls into
            one 4-bank psum tile at row bases {0,32,64,96} -> 4 direct
            psum->SBUF row DMAs into qk_T [88,N] f32
  P4 tm:    PE-transpose qk_T -> qk_tm token-major bf16
  P5 feat:  (per token-half) tap-gathers, feat=elu(qk+b)+1, A=EC*feat/z
  P6 A_T:   (per half) PE-transpose A_tm (f32) -> A_stage [88,N] bf16
  P7 AV:    (per nb) per (pair,off): sel-matmul expand, DVE mul w/ padded v2,
            PE identity accumulate
  P8 out:   (per nb) gg = out_fm*gate; y_fm = Wout^T.T @ gg (PE bf16) -> DRAM
Host: shard, pre-transpose weights to bf16, build sel/tap tables, reduce
head-halves.
"""
from contextlib import ExitStack

import ml_dtypes
import numpy as np

import concourse.bass as bass
import concourse.mybir as mybir
import concourse.tile as tile
from concourse import bacc
from concourse.bass_utils import run_bass_kernel_spmd
from concourse.masks import make_identity

F32 = mybir.dt.float32
BF16 = mybir.dt.bfloat16
AF = mybir.ActivationFunctionType
ALU = mybir.AluOpType
AX = mybir.AxisListType

B, N, D, H = 4, 2048, 1024, 16
HD = 64
NS = 11
HL = 8
NPAIR = 4
PAD = 1536
NPADCOLS = PAD + N
D4 = [0.4829629131445341, 0.8365163037378079, 0.2241438680420134, -0.1294095225512604]

TAPS = []
for _j in range(NS):
    _d = 1 << _j
    for _tau in range(4):
        _off = _d * _tau
        if _off != 0 and _off >= N:
            continue
        TAPS.append((_j, _tau, _off, 4 * _j + _tau))
NTAP = len(TAPS)            # 42
NTB = 44                    # full (j, tau) grid; invalid slots get coef 0
OFFSETS = sorted({t[2] for t in TAPS})
NOFF = len(OFFSETS)         # 22
OFF_IDX = {o: i for i, o in enumerate(OFFSETS)}
NCH = N // 128
NNB = N // 512

_KERNEL_CACHE = {}


def build_kernel(dbg=False):
    nc = bacc.Bacc("TRN2", target_bir_lowering=False, debug=False, num_devices=8)

    xT = nc.dram_tensor("xT", [D, N], BF16, kind="ExternalInput")
    wT = nc.dram_tensor("wT", [D, 4 * HL * HD], BF16, kind="ExternalInput")
    woT = nc.dram_tensor("woT", [HL * HD, D], BF16, kind="ExternalInput")
    wqs = nc.dram_tensor("wqs", [128, 2 * NS], BF16, kind="ExternalInput")
    sel_c = nc.dram_tensor("sel_c", [128, 2 * NOFF * 128], BF16,
                           kind="ExternalInput")
    prior_c = nc.dram_tensor("prior_c", [128, HL * NS], F32, kind="ExternalInput")
    bias_c = nc.dram_tensor("bias_c", [128, HL * NTB], F32, kind="ExternalInput")
    coef_c = nc.dram_tensor("coef_c", [128, HL * NTB], F32, kind="ExternalInput")
    bqkv_c = nc.dram_tensor("bqkv_c", [128, 12], F32, kind="ExternalInput")
    bgate_c = nc.dram_tensor("bgate_c", [128, NPAIR], F32, kind="ExternalInput")
    bp_c = nc.dram_tensor("bp_c", [128, HL], F32, kind="ExternalInput")

    y_fm = nc.dram_tensor("y_fm", [D, N], F32, kind="ExternalOutput")
    if dbg:
        d_q2 = nc.dram_tensor("d_q2", [128, N], BF16, kind="ExternalOutput")
        d_k2 = nc.dram_tensor("d_k2", [128, NPADCOLS], BF16,
                              kind="ExternalOutput")
        d_v2 = nc.dram_tensor("d_v2", [128, NPADCOLS], BF16,
                              kind="ExternalOutput")
        d_gate = nc.dram_tensor("d_gate", [128, N], BF16, kind="ExternalOutput")
        d_qkT = nc.dram_tensor("d_qkT", [88, N], BF16, kind="ExternalOutput")
        d_E = nc.dram_tensor("d_E", [128, NCH * HL * NS], BF16,
                             kind="ExternalOutput")
        d_qktm = nc.dram_tensor("d_qktm", [128, NCH * 2 * 88], BF16,
                                kind="ExternalOutput")
        d_ast = nc.dram_tensor("d_ast", [88, N], BF16, kind="ExternalOutput")
        d_ofm = nc.dram_tensor("d_ofm", [128, N], BF16, kind="ExternalOutput")

    CH = NCH * HL               # 128 (c,h) groups
    W = NCH * HL * NTB          # 5632

    with tile.TileContext(nc) as tc, ExitStack() as S:
        # ---- persistent pools ----
        const = S.enter_context(tc.tile_pool(name="const", bufs=1))
        big = S.enter_context(tc.tile_pool(name="big", bufs=1, side="right"))
        k2 = [big.tile([128, NPADCOLS], BF16, tag=f"k2_{p}", name=f"k2_{p}")
              for p in range(NPAIR)]
        v2 = [big.tile([128, NPADCOLS], BF16, tag=f"v2_{p}", name=f"v2_{p}")
              for p in range(NPAIR)]
        gate = [big.tile([128, N], BF16, tag=f"g_{p}", name=f"g_{p}")
                for p in range(NPAIR)]
        out_fm = [big.tile([128, N], BF16, tag=f"o_{p}", name=f"o_{p}")
                  for p in range(NPAIR)]

        S_as = ExitStack()
        arow = S_as.enter_context(tc.tile_pool(name="arow", bufs=1, side="right"))
        A_stage = [arow.tile([88, N], BF16, tag=f"ast{t}", name=f"ast{t}")
                   for t in range(2)]

        S_e = ExitStack()
        qk_Tp = S_e.enter_context(tc.tile_pool(name="qkTp", bufs=1, side="right"))
        qk_T = [qk_Tp.tile([88, N], BF16, tag=f"qkT{t}", name=f"qkT{t}")
                for t in range(2)]
        epre = S_e.enter_context(tc.tile_pool(name="epre", bufs=1))
        E_tm = epre.tile([128, NCH * HL * NS], BF16)
        SE_tm = epre.tile([128, NCH * HL], F32)
        EC_t = epre.tile([128, W], BF16)
        qk_tm = epre.tile([128, NCH * 2 * 88], BF16, name="qk_tm")
        wqs_r = const.tile([128, 2 * NS], BF16)
        prior_t = const.tile([128, HL * NS], F32)
        bp_t = const.tile([128, HL], F32)

        # ======== P1: projections (input DMAs first for fast start) ========
        S_q = ExitStack()
        qp = S_q.enter_context(tc.tile_pool(name="qp", bufs=1, side="right"))
        q2 = [qp.tile([128, N], BF16, tag=f"q2_{p}", name=f"q2_{p}")
              for p in range(NPAIR)]
        bqkv_t = const.tile([128, 12], F32)
        bgate_t = const.tile([128, NPAIR], F32)
        ident_f = const.tile([128, 128], F32)
        ident_b = const.tile([128, 128], BF16)
        ones_f = const.tile([128, 2], F32)
        ones2 = const.tile([128, 2], BF16)
        bias_t = const.tile([128, HL * NTB], F32)
        coef_t = const.tile([128, HL * NTB], F32)
        def p2_unit(c, psp, scp):
            ps = psp.tile([128, HL * NS], F32, tag="qs")
            for p in range(NPAIR):
                nc.tensor.matmul(
                    ps[:, 2 * NS * p:2 * NS * (p + 1)],
                    q2[p][:, 128 * c:128 * (c + 1)], wqs_r[:],
                    start=True, stop=True)
            et = scp.tile([128, HL * NS], F32, tag="et")
            nc.vector.tensor_tensor(out=et[:], in0=ps[:], in1=prior_t[:],
                                    op=ALU.add)
            nc.scalar.activation(E_tm[:, HL * NS * c:HL * NS * (c + 1)],
                                 et[:], AF.Exp)

        def p4_unit(ct, psp):
            c, t = ct // 2, ct % 2
            ps = psp.tile([128, 88], BF16, tag="tp")
            nc.tensor.transpose(
                ps[:], qk_T[t][:, 128 * c:128 * (c + 1)],
                ident_b[0:88, 0:88])
            nc.scalar.copy(
                qk_tm[:, 176 * c + 88 * t:176 * c + 88 * (t + 1)], ps[:])

        def p3_unit(oi, off, prp, stp, psp):
            prods = []
            oc = off if off >= 128 else 0   # zero-region worth skipping
            for p in range(NPAIR):
                prod = prp.tile([128, N], BF16, tag=f"prod{p}")
                if oc:
                    nc.gpsimd.memset(prod[:, 0:oc].bitcast(F32), 0.0)
                nc.vector.tensor_tensor(
                    out=prod[:, oc:N], in0=q2[p][:, oc:N],
                    in1=k2[p][:, PAD - off + oc:PAD - off + N], op=ALU.mult)
                prods.append(prod)
            ps = psp.tile([128, N], F32, tag="qkps")
            for nb in range(NNB):
                for p in range(NPAIR):
                    nc.tensor.matmul(
                        ps[32 * p:32 * p + 2, 512 * nb:512 * (nb + 1)],
                        ones2[:], prods[p][:, 512 * nb:512 * (nb + 1)],
                        start=True, stop=True,
                        tile_position=(0, 32 * p))
            st = stp.tile([128, N], BF16, tag="stage")
            nc.scalar.copy(st[:], ps[:])
            for t in range(2):
                for ph in range(2):
                    p = 2 * t + ph
                    dst = qk_T[t][:].rearrange(
                        "(a j o) n -> a j o n", a=2, j=2)[ph, :, oi, :]
                    nc.sync.dma_start(dst, st[32 * p:32 * p + 2, :])

        with tc.tile_pool(name="p1x", bufs=1) as p1x, \
             tc.tile_pool(name="p1", bufs=2) as p1, \
             tc.tile_pool(name="p3a", bufs=1) as p3a, \
             tc.tile_pool(name="p3sa", bufs=2) as p3sa, \
             tc.tile_pool(name="p2sc", bufs=2) as p2sc, \
             tc.tile_pool(name="p3psa", bufs=1, space="PSUM") as p3psa, \
             tc.tile_pool(name="p2psa", bufs=1, space="PSUM") as p2psa, \
             tc.tile_pool(name="p1ps", bufs=3, space="PSUM") as p1ps:
            def load_wr(fc):
                wrt = p1.tile([128, 8 * 128], BF16, tag="wr")
                nc.sync.dma_start(
                    wrt[:].rearrange("p (a m) -> p a m", a=8),
                    wT[:, 128 * fc:128 * (fc + 1)]
                    .rearrange("(a p) m -> p a m", p=128))
                return wrt

            wr_pre = [load_wr(0)]
            nc.sync.dma_start(bqkv_t[:], bqkv_c[:])
            nc.sync.dma_start(bgate_t[:], bgate_c[:])
            nc.sync.dma_start(bias_t[:], bias_c[:])
            nc.sync.dma_start(coef_t[:], coef_c[:])
            nc.sync.dma_start(wqs_r[:], wqs[:])
            nc.sync.dma_start(prior_t[:], prior_c[:])
            nc.sync.dma_start(bp_t[:], bp_c[:])
            make_identity(nc, ident_f)
            nc.vector.tensor_copy(ident_b[:], ident_f[:])
            nc.vector.memset(ones_f[:], 0.0)
            nc.vector.memset(ones_f[0:64, 0:1], 1.0)
            nc.vector.memset(ones_f[64:128, 1:2], 1.0)
            nc.vector.tensor_copy(ones2[:], ones_f[:])
            xr = []
            for kc in range(D // 128):
                xrt = p1x.tile([128, N], BF16, tag=f"xr{kc}", name=f"xr{kc}")
                nc.sync.dma_start(xrt[:], xT[128 * kc:128 * (kc + 1), :])
                xr.append(xrt)
            for p in range(NPAIR):
                nc.vector.memset(k2[p][:, 0:PAD], 0.0)
                nc.vector.memset(v2[p][:, 0:PAD], 0.0)
            for fc in range(16):
                wr = wr_pre[fc] if fc < len(wr_pre) else load_wr(fc)
                sect, pair = fc // 4, fc % 4
                for nb in range(NNB):
                    ps = p1ps.tile([128, 512], F32, tag="proj")
                    for kc in range(D // 128):
                        nc.tensor.matmul(
                            ps[:], wr[:, 128 * kc:128 * (kc + 1)],
                            xr[kc][:, 512 * nb:512 * (nb + 1)],
                            start=(kc == 0), stop=(kc == 7))
                    sl = slice(512 * nb, 512 * (nb + 1))
                    slp = slice(PAD + 512 * nb, PAD + 512 * (nb + 1))
                    if sect == 0:
                        nc.scalar.activation(q2[pair][:, sl], ps[:],
                                             AF.Identity, bias=bqkv_t[:, fc:fc + 1])
                    elif sect == 1:
                        nc.scalar.activation(k2[pair][:, slp], ps[:],
                                             AF.Identity, bias=bqkv_t[:, fc:fc + 1])
                    elif sect == 2:
                        nc.scalar.activation(v2[pair][:, slp], ps[:],
                                             AF.Identity, bias=bqkv_t[:, fc:fc + 1])
                    else:
                        nc.scalar.activation(gate[pair][:, sl], ps[:], AF.Sigmoid,
                                             bias=bgate_t[:, pair:pair + 1])
                if 4 <= fc < 8:
                    for u in range(4):
                        p2_unit(4 * (fc - 4) + u, p2psa, p2sc)
                elif fc >= 8:
                    sched = (3 * (fc - 8) if fc < 14
                             else 18 + 2 * (fc - 14))
                    cnt = 3 if fc < 14 else 2
                    for u in range(cnt):
                        uoi = sched + u
                        p3_unit(uoi, OFFSETS[uoi], p3a, p3sa, p3psa)

        # ---- deferred allocations (land in space freed by P1 transients) ----
        ep = S_e.enter_context(tc.tile_pool(name="ep", bufs=1))
        sel_t = ep.tile([128, 2 * NOFF * 128], BF16)
        nc.sync.dma_start(sel_t[:], sel_c[:])

        nc.vector.tensor_reduce(
            SE_tm[:].rearrange("p (c h) -> p c h", h=HL).unsqueeze(-1),
            E_tm[:].rearrange("p (c h s) -> p c h s", h=HL, s=NS),
            axis=AX.X, op=ALU.add)


        if dbg:
            nc.sync.dma_start(d_q2[:], q2[0][:])
            nc.sync.dma_start(d_k2[:], k2[0][:])
            nc.sync.dma_start(d_v2[:], v2[0][:])
            nc.sync.dma_start(d_gate[:], gate[0][:])

        S_q.close()  # q2 released
        if dbg:
            nc.sync.dma_start(d_qkT[:], qk_T[0][:])
            nc.sync.dma_start(d_E[:], E_tm[:])

        if dbg:
            nc.sync.dma_start(d_qktm[:], qk_tm[:])

        # ======== P4 standalone ========
        with tc.tile_pool(name="p4ps", bufs=4, space="PSUM") as p4ps:
            for ct in range(2 * NCH):
                p4_unit(ct, p4ps)

        # ======== P5..P8 pipelined by token halves / nb blocks ========
        with tc.tile_pool(name="p5", bufs=2) as p5, \
             tc.tile_pool(name="p6ps", bufs=1, space="PSUM") as p6ps, \
             tc.tile_pool(name="p7", bufs=8) as p7, \
             tc.tile_pool(name="p7ps", bufs=3, space="PSUM") as p7ps, \
             tc.tile_pool(name="p7po", bufs=2, space="PSUM") as p7po, \
             tc.tile_pool(name="p8", bufs=2) as p8, \
             tc.tile_pool(name="p8g", bufs=1) as p8g, \
             tc.tile_pool(name="p8ps", bufs=2, space="PSUM") as p8ps:
            wo_r = []
            for p in range(NPAIR):
                wor = p8g.tile([128, D], BF16, tag=f"wor{p}", name=f"wor{p}")
                nc.sync.dma_start(wor[:], woT[128 * p:128 * (p + 1), :])
                wo_r.append(wor)
            sel_v = sel_t[:].rearrange("p (i m) -> p i m", i=2 * NOFF)

            def p5_half(hf):
                c0, c1 = 4 * hf, 4 * hf + 4   # chunk range (one nb block)
                geng = nc.gpsimd
                ncc = c1 - c0                 # 8 chunks
                Wh = ncc * HL * NTB           # 2816
                CHh = ncc * HL
                qk_s = qk_tm[:, 176 * c0:176 * c1]
                qk8 = qk_s.rearrange("p (ch o) -> p ch o", o=NOFF)
                qk9 = qk_s.rearrange("p (ch e two) -> p ch e two", e=NS, two=2)
                EC_s = EC_t[:, HL * NTB * c0:HL * NTB * c1]
                SE_s = SE_tm[:, HL * c0:HL * c1]

                y_t = p5.tile([128, Wh], BF16, tag="y")
                y7 = y_t[:].rearrange("p (ch j f) -> p ch j f", j=NS, f=4)
                geng.tensor_copy(
                    y7[:, :, :, 0:1],
                    qk8[:, :, 0:1].unsqueeze(2).broadcast_to([128, CHh, NS, 1]))
                geng.tensor_copy(y7[:, :, 0:1, 1:2],
                                      qk8[:, :, 1:2].unsqueeze(2))
                geng.tensor_copy(y7[:, :, 1:2, 1:2],
                                      qk8[:, :, 2:3].unsqueeze(2))
                geng.tensor_copy(y7[:, :, 2:11, 1:2],
                                      qk9[:, :, 2:11, 0:1])
                geng.tensor_copy(y7[:, :, 0:1, 2:3],
                                      qk8[:, :, 2:3].unsqueeze(2))
                geng.tensor_copy(y7[:, :, 1:10, 2:3],
                                      qk9[:, :, 2:11, 0:1])
                geng.tensor_copy(y7[:, :, 10:11, 2:3],
                                      qk9[:, :, 10:11, 0:1])
                geng.tensor_copy(y7[:, :, 0:10, 3:4],
                                      qk9[:, :, 1:11, 1:2])
                geng.tensor_copy(y7[:, :, 10:11, 3:4],
                                      qk9[:, :, 10:11, 1:2])
                nc.gpsimd.tensor_copy(
                    EC_s.rearrange("p (ch s f) -> p ch s f", s=NS, f=4),
                    E_tm[:, HL * NS * c0:HL * NS * c1]
                    .rearrange("p (ch s) -> p ch s", s=NS)
                    .unsqueeze(-1).broadcast_to([128, CHh, NS, 4]))
                nc.vector.tensor_tensor(
                    out=EC_s.rearrange("p (c w) -> p c w", w=HL * NTB),
                    in0=EC_s.rearrange("p (c w) -> p c w", w=HL * NTB),
                    in1=coef_t[:].unsqueeze(1)
                    .broadcast_to([128, ncc, HL * NTB]),
                    op=ALU.mult)

                nc.vector.tensor_tensor(
                    out=y_t[:].rearrange("p (c w) -> p c w", w=HL * NTB),
                    in0=y_t[:].rearrange("p (c w) -> p c w", w=HL * NTB),
                    in1=bias_t[:].unsqueeze(1).broadcast_to([128, ncc, HL * NTB]),
                    op=ALU.add)
                m0 = p5.tile([128, Wh], BF16, tag="m0")
                nc.vector.tensor_scalar(out=m0[:], in0=y_t[:], scalar1=0.0,
                                        scalar2=None, op0=ALU.min)
                nc.scalar.activation(m0[:], m0[:], AF.Exp)
                nc.vector.tensor_scalar(out=y_t[:], in0=y_t[:], scalar1=0.0,
                                        scalar2=None, op0=ALU.max)
                nc.vector.tensor_tensor(out=m0[:], in0=m0[:], in1=y_t[:],
                                        op=ALU.add)
                nc.vector.tensor_tensor(out=m0[:], in0=m0[:], in1=EC_s,
                                        op=ALU.mult)

                ab = p5.tile([128, CHh], F32, tag="ab")
                t0 = p5.tile([128, CHh], F32, tag="t0")
                nc.vector.tensor_scalar(out=ab[:], in0=qk8[:, :, 0:1].squeeze(-1),
                                        scalar1=0.0, scalar2=None, op0=ALU.min)
                nc.scalar.activation(ab[:], ab[:], AF.Exp)
                nc.vector.tensor_scalar(out=t0[:], in0=qk8[:, :, 0:1].squeeze(-1),
                                        scalar1=0.0, scalar2=None, op0=ALU.max)
                nc.vector.tensor_tensor(out=ab[:], in0=ab[:], in1=t0[:],
                                        op=ALU.add)
                nc.vector.tensor_tensor(out=ab[:], in0=ab[:], in1=SE_s,
                                        op=ALU.mult)
                nc.vector.tensor_tensor(
                    out=ab[:].rearrange("p (c h) -> p c h", h=HL),
                    in0=ab[:].rearrange("p (c h) -> p c h", h=HL),
                    in1=bp_t[:].unsqueeze(1).broadcast_to([128, ncc, HL]),
                    op=ALU.mult)

                z_t = p5.tile([128, CHh], F32, tag="z")
                nc.vector.tensor_reduce(
                    z_t[:].rearrange("p (c h) -> p c h", h=HL).unsqueeze(-1),
                    m0[:].rearrange("p (c h t) -> p c h t", h=HL, t=NTB),
                    axis=AX.X, op=ALU.add, apply_absolute_value=True)
                nc.vector.tensor_tensor(out=z_t[:], in0=z_t[:], in1=ab[:],
                                        op=ALU.add)
                nc.vector.scalar_tensor_tensor(
                    out=z_t[:], in0=SE_s, scalar=1e-6, in1=z_t[:],
                    op0=ALU.mult, op1=ALU.add)
                nc.vector.reciprocal(z_t[:], z_t[:])
                nc.vector.tensor_tensor(
                    out=m0[:].rearrange("p (ch t) -> p ch t", t=NTB),
                    in0=m0[:].rearrange("p (ch t) -> p ch t", t=NTB),
                    in1=z_t[:].unsqueeze(-1).broadcast_to([128, CHh, NTB]),
                    op=ALU.mult)
                nc.vector.tensor_tensor(out=ab[:], in0=ab[:], in1=z_t[:],
                                        op=ALU.mult)

                A_tm = p5.tile([128, ncc * HL * NOFF], F32, tag="atm")
                A8 = A_tm[:].rearrange("p (ch o) -> p ch o", o=NOFF)
                A9 = A_tm[:].rearrange("p (ch e two) -> p ch e two", e=NS, two=2)
                m7 = m0[:].rearrange("p (ch j f) -> p ch j f", j=NS, f=4)
                nc.vector.tensor_reduce(A8[:, :, 0:1].unsqueeze(-1),
                                        m7[:, :, :, 0:1].transpose([0, 1, 3, 2]),
                                        axis=AX.X, op=ALU.add)
                nc.vector.tensor_tensor(out=A8[:, :, 0:1].squeeze(-1),
                                        in0=A8[:, :, 0:1].squeeze(-1),
                                        in1=ab[:], op=ALU.add)
                nc.vector.tensor_copy(A8[:, :, 1:2],
                                      m7[:, :, 0:1, 1:2].squeeze(-1))
                nc.vector.tensor_tensor(out=A9[:, :, 1:11, 0:1],
                                        in0=m7[:, :, 1:11, 1:2],
                                        in1=m7[:, :, 0:10, 2:3], op=ALU.add)
                nc.vector.tensor_copy(A9[:, :, 1:11, 1:2],
                                      m7[:, :, 0:10, 3:4])

                # P6: transpose to A_stage columns of this half
                for ci in range(ncc):
                    c = c0 + ci
                    for t in range(2):
                        ps2 = p6ps.tile([88, 128], F32, tag="tpb")
                        nc.tensor.transpose(
                            ps2[:],
                            A_tm[:, 176 * ci + 88 * t:176 * ci + 88 * (t + 1)],
                            ident_f[:])
                        nc.scalar.copy(A_stage[t][:, 128 * c:128 * (c + 1)],
                                       ps2[:])

            def p7_block(nb):
                n0 = 512 * nb
                for p in range(NPAIR):
                    t, ph = p // 2, p % 2
                    po = p7po.tile([128, 512], F32, tag="avo")
                    valid = [(oi, off) for oi, off in enumerate(OFFSETS)
                             if off < n0 + 512]
                    for vi, (oi, off) in enumerate(valid):
                        pa = p7ps.tile([128, 512], F32, tag="aexp")
                        nc.tensor.matmul(
                            pa[:], sel_v[0:88, NOFF * ph + oi, :],
                            A_stage[t][:, n0:n0 + 512],
                            start=True, stop=True)
                        tmp = p7.tile([128, 512], BF16, tag="avt")
                        nc.vector.tensor_tensor(
                            out=tmp[:],
                            in0=v2[p][:, PAD + n0 - off:PAD + n0 + 512 - off],
                            in1=pa[:], op=ALU.mult)
                        nc.tensor.matmul(
                            po[:], ident_b[:], tmp[:],
                            start=(vi == 0), stop=(vi == len(valid) - 1))
                    nc.scalar.copy(out_fm[p][:, n0:n0 + 512], po[:])

            def p8_block(nb):
                n0 = 512 * nb
                for p in range(NPAIR):
                    nc.gpsimd.tensor_tensor(
                        out=out_fm[p][:, n0:n0 + 512],
                        in0=out_fm[p][:, n0:n0 + 512],
                        in1=gate[p][:, n0:n0 + 512], op=ALU.mult)
                for dc in range(D // 128):
                    ps = p8ps.tile([128, 512], F32, tag="yps")
                    for p in range(NPAIR):
                        nc.tensor.matmul(
                            ps[:], wo_r[p][:, 128 * dc:128 * (dc + 1)],
                            out_fm[p][:, n0:n0 + 512],
                            start=(p == 0), stop=(p == NPAIR - 1))
                    yt = p8.tile([128, 512], F32, tag="yt")
                    nc.scalar.copy(yt[:], ps[:])
                    nc.sync.dma_start(
                        y_fm[128 * dc:128 * (dc + 1), n0:n0 + 512], yt[:])

            p5_half(0)
            p7_block(0)
            p5_half(1)
            p8_block(0)
            p7_block(1)
            p5_half(2)
            p8_block(1)
            p7_block(2)
            p5_half(3)
            if dbg:
                nc.sync.dma_start(d_ast[:], A_stage[0][:])
            p8_block(2)
            p7_block(3)
            p8_block(3)
            if dbg:
                nc.sync.dma_start(d_ofm[:], out_fm[0][:])

        S_e.close()
        S_as.close()
    nc.compile()
    return nc


# ===========================================================================
# host side
# ===========================================================================

_SEL = np.zeros((128, 2 * NOFF * 128), np.float32)
for _ph in range(2):
    for _oi in range(NOFF):
        _i = NOFF * _ph + _oi
        _SEL[44 * _ph + _oi, 128 * _i:128 * _i + 64] = 1.0
        _SEL[44 * _ph + NOFF + _oi, 128 * _i + 64:128 * (_i + 1)] = 1.0


def _bf16(a):
    return np.ascontiguousarray(a).astype(ml_dtypes.bfloat16)


def _make_inputs(x, W_qkv, b_qkv, W_out, W_gate, b_gate, scale_gain, W_qscale,
                 identity_bypass, pos_bias, b, g):
    hg0 = g * HL
    rows = slice(hg0 * HD, (hg0 + HL) * HD)
    Wq = W_qkv[0 * D:1 * D][rows]
    Wk = W_qkv[1 * D:2 * D][rows]
    Wv = W_qkv[2 * D:3 * D][rows]
    Wg = W_gate[rows]
    wTv = np.concatenate([Wq, Wk, Wv, Wg], axis=0).T.copy()
    woTv = W_out[:, rows].T.copy()

    wqsv = np.zeros((128, 2 * NS), np.float32)
    wqsv[0:64, 0:NS] = W_qscale.T
    wqsv[64:128, NS:2 * NS] = W_qscale.T

    prior = np.zeros((HL, NS), np.float32)
    for h in range(HL):
        prior[h] = scale_gain[:, hg0 + h]
    prior_v = np.broadcast_to(prior.reshape(1, -1), (128, HL * NS)).copy()

    bias = np.zeros((HL, NTB), np.float32)
    coef = np.zeros((HL, NTB), np.float32)
    for h in range(HL):
        for (j, tau, off, full_idx) in TAPS:
            bias[h, full_idx] = pos_bias[full_idx, hg0 + h]
            coef[h, full_idx] = D4[tau]
    bias_v = np.broadcast_to(bias.reshape(1, -1), (128, HL * NTB)).copy()
    coef_v = np.broadcast_to(coef.reshape(1, -1), (128, HL * NTB)).copy()

    bqkv = np.zeros((128, 12), np.float32)
    for sect, bb in enumerate([b_qkv[0:D], b_qkv[D:2 * D], b_qkv[2 * D:3 * D]]):
        sl = bb[rows]
        for pair in range(NPAIR):
            bqkv[:, sect * 4 + pair] = sl[128 * pair:128 * (pair + 1)]
    bgate_v = np.zeros((128, NPAIR), np.float32)
    gsl = b_gate[rows]
    for pair in range(NPAIR):
        bgate_v[:, pair] = gsl[128 * pair:128 * (pair + 1)]

    bp = np.log1p(np.exp(identity_bypass[hg0:hg0 + HL])).astype(np.float32)
    bp_v = np.broadcast_to(bp.reshape(1, -1), (128, HL)).copy()

    return {
        "xT": _bf16(x[b].T),
        "wT": _bf16(wTv),
        "woT": _bf16(woTv),
        "wqs": _bf16(wqsv),
        "sel_c": _bf16(_SEL),
        "prior_c": np.ascontiguousarray(prior_v),
        "bias_c": np.ascontiguousarray(bias_v),
        "coef_c": np.ascontiguousarray(coef_v),
        "bqkv_c": bqkv,
        "bgate_c": bgate_v,
        "bp_c": np.ascontiguousarray(bp_v),
    }


def kernel(x, W_qkv, b_qkv, W_out, b_out, W_gate, b_gate, scale_gain, W_qscale,
           identity_bypass, pos_bias):
    x = np.asarray(x, np.float32)
    args = [np.asarray(a, np.float32) for a in
            (W_qkv, b_qkv, W_out, W_gate, b_gate, scale_gain, W_qscale,
             identity_bypass, pos_bias)]
    (W_qkv, b_qkv, W_out, W_gate, b_gate, scale_gain, W_qscale,
     identity_bypass, pos_bias) = args

    if "nc" not in _KERNEL_CACHE:
        _KERNEL_CACHE["nc"] = build_kernel()
    nc = _KERNEL_CACHE["nc"]

    in_maps = []
    for core in range(8):
        b, g = core % 4, core // 4
        in_maps.append(_make_inputs(x, W_qkv, b_qkv, W_out, W_gate, b_gate,
                                    scale_gain, W_qscale, identity_bypass,
                                    pos_bias, b, g))
    res = run_bass_kernel_spmd(nc, in_maps, list(range(8)))

    out = np.zeros((B, N, D), np.float32)
    for core in range(8):
        b = core % 4
        out[b] += res.results[core]["y_fm"].T
    out += np.asarray(b_out, np.float32)
    return out
